# revision 41
# baseline (speedup 1.0000x reference)
"""Trainium2 Bass kernel for nn_BinaryTokenClassificationModel (segment_reduce).

Math: the reference pools token embeddings into word embeddings (mean over
contiguous runs of equal word ids), then computes
    logits[b,s,t] = src_pooled[b,s] @ w_src + tgt_pooled[b,t] @ w_tgt + b.
Because the classifier is linear, pooling and projection commute:
    u_src[t] = tok_h[t] @ w_src,  u_tgt[t] = tok_h[t] @ w_tgt
    logits[s,t] = sum_p atw_src[p,s] u_src[p] + sum_p atw_tgt[p,t] u_tgt[p] + b
where atw is the 1/count-weighted segment-membership matrix.

Device plan (per core = one batch row, no collectives):
  - Host ships ONE [128, ncol] bf16 blob in two sync-ring DMA pieces with
    ~3.1-3.4 KB per-row descriptors (the per-queue descriptor stream runs at
    ~24 GB/s only for rows of this size). Piece 1 = chunks 2,3 token data
    pre-transposed in 128-blocks (tokT[p,t] = tok[c*128+t, j*128+p]) plus ALL
    metadata appended to the same rows: 12 interleaved w_src/w_tgt column
    blocks, an aw-wide iota row, and 9 bf16 scalars (bias, per-chunk segment
    column idx, per-chunk 1/count weight). Piece 2 = chunks 0,1 tokens.
    Separate small-row metadata DMAs are ~10x slower (descriptor-bound) and
    contend with the token stream - everything rides the two big pieces.
  - Projections run on the TensorEngine: per chunk six accumulating K=128
    matmuls (tokT block x w column) into a PSUM f32 [128,1]; DVE casts to
    SBUF bf16.
  - atw matrices are built ON DEVICE: DVE casts the bf16 idx/wgt scalars to
    f32 once, then one tensor_scalar(is_equal, mult) per chunk:
    atw[p,w] = (iota[w]==idx[p]) * wgt[p]. No dense membership DMA.
  - One more TensorE matmul per chunk (two in the general layout) accumulates
    pooling + outer-sum into the [S,T] PSUM tile, broadcasting the u column
    along the free dim.
  - DVE adds the bias column during the PSUM->SBUF copy; the sync ring
    stores the fp32 output. ScalarE is never used -> no ACT table load.
"""

import functools

import ml_dtypes
import numpy as np

import concourse.bacc as bacc
import concourse.mybir as mybir
from concourse.bass_utils import run_bass_kernel_spmd
from concourse.tile import TileContext

# Problem geometry (hardcoded per spec)
B = 8
L_SRC = 256
L_TGT = 256
L = L_SRC + L_TGT  # 512
H = 768
P = 128            # SBUF partitions / tokens per chunk
NCHUNK = L // P    # 4
NBLK = H // P      # 6
N_SRC_CHUNKS = L_SRC // P  # 2
N_CORES = 8
F32 = mybir.dt.float32
BF16 = mybir.dt.bfloat16

ORDER = (3, 2, 0, 1)   # piece arrival order: each DMA engine services the
                       # sync ring's chain before the act ring's, so with
                       # sync=[meta+c3, c0] and act=[c2, c1] chunks arrive
                       # exactly in ORDER, metadata first
NW_META = 12           # interleaved w blocks: col 2j = w_src_j, 2j+1 = w_tgt_j
NF_META = 9            # bf16 scalars: bias, idx x4 (ORDER), wgt x4 (ORDER)


def _cols(aw):
    """Column offsets within the blob: [meta | tok chunks in ORDER].
    The metadata (W columns, iota, scalars) rides the FIRST-serviced piece
    so projections can begin the moment the first token chunk lands."""
    w0 = 0                     # 12 interleaved w block columns
    iota0 = w0 + NW_META
    meta0 = iota0 + aw
    tok0 = meta0 + NF_META     # chunks in ORDER sequence
    ncol = tok0 + 4 * H
    return w0, iota0, meta0, tok0, ncol


# ---------------------------------------------------------------------------
# Host-side segment bookkeeping (exact mirror of reference._pool_words)
# ---------------------------------------------------------------------------

def _segments(combined_wid, attention_mask, n_words):
    """Per-token dense run ids exactly as the reference computes them."""
    valid = (attention_mask > 0) & (combined_wid >= 0)  # [B, L]
    prev_wid = np.concatenate(
        [np.full((combined_wid.shape[0], 1), -2, dtype=combined_wid.dtype),
         combined_wid[:, :-1]], axis=1)
    prev_valid = np.concatenate(
        [np.zeros((valid.shape[0], 1), dtype=bool), valid[:, :-1]], axis=1)
    new_run = valid & ((combined_wid != prev_wid) | (~prev_valid))
    run_id = np.cumsum(new_run.astype(np.int64), axis=1) - 1  # [B, L]
    seg = np.where(valid, run_id, n_words)  # n_words = dummy slot
    return seg, valid


def _seg_weights(seg, valid, n_words):
    """1/max(count,1) weight for each token's segment (0 for invalid)."""
    Bv, Lv = seg.shape
    wgt = np.zeros((Bv, Lv), dtype=np.float32)
    for b in range(Bv):
        counts = np.bincount(seg[b][valid[b]], minlength=Lv + 1).astype(np.float32)
        inv = 1.0 / np.maximum(counts, 1.0)
        wgt[b] = np.where(valid[b] & (seg[b] < n_words), inv[np.minimum(seg[b], Lv)], 0.0)
    return wgt


# ---------------------------------------------------------------------------
# Device kernel
# ---------------------------------------------------------------------------

def _emit_body(nc, tc, S, T, aw):
    """aw = atw column width (P for the block layout where src-chunk tokens
    pool into [0,S) and tgt-chunk tokens into [S,S+T); S+T for the general
    layout where any token may pool anywhere and both projections are kept).
    """
    general = aw != P
    w0, iota0, meta0, tok0, ncol = _cols(aw)
    blob_d = nc.declare_dram_parameter("blob", [P, ncol], BF16, isOutput=False)
    out_d = nc.declare_dram_parameter("out", [S, T], F32, isOutput=True)

    EQ = mybir.AluOpType.is_equal
    MUL = mybir.AluOpType.mult

    with (
        tc.tile_pool(name="sb", bufs=1) as sbp,
        tc.tile_pool(name="psum", bufs=1, space="PSUM") as psp,
    ):
        blob_sb = sbp.tile([P, ncol], BF16)

        # input split across BOTH HWDGE rings; each DMA engine services the
        # sync ring's chain before the act ring's, so pieces arrive in
        # emission order below: meta+ORDER[0], ORDER[1], ORDER[2], ORDER[3]
        b1 = tok0 + H
        b2 = tok0 + 2 * H
        b3 = tok0 + 3 * H
        nc.sync.dma_start(out=blob_sb[:, 0:b1], in_=blob_d[:, 0:b1])
        nc.scalar.dma_start(out=blob_sb[:, b1:b2], in_=blob_d[:, b1:b2])
        nc.sync.dma_start(out=blob_sb[:, b2:b3], in_=blob_d[:, b2:b3])
        nc.scalar.dma_start(out=blob_sb[:, b3:ncol], in_=blob_d[:, b3:ncol])

        iota_sb = blob_sb[:, iota0:iota0 + aw]

        # bf16 meta scalars -> f32 (is_equal wants f32 per-partition scalars)
        # and the atw builds run at high priority so the DVE does them the
        # moment piece 1 lands instead of queueing them behind the u casts
        with tc.high_priority():
            meta_f = sbp.tile([P, NF_META], F32)
            nc.vector.tensor_copy(meta_f[:], blob_sb[:, meta0:meta0 + NF_META])

            # atw[p, w] = (w == idx_c[p]) * wgt_c[p], one DVE op per chunk
            atw = []
            for oi in range(NCHUNK):
                a = sbp.tile([P, aw], BF16, name=f"atw_{oi}")
                nc.vector.tensor_scalar(
                    out=a[:], in0=iota_sb,
                    scalar1=meta_f[:, 1 + oi:2 + oi],
                    scalar2=meta_f[:, 5 + oi:6 + oi],
                    op0=EQ, op1=MUL)
                atw.append(a)

        nw_u = 2 if general else 1
        u_sb = sbp.tile([P, NCHUNK * nw_u], BF16)
        psum_out = psp.tile([S, T], F32)

        # chunk c -> its tokT column offset (chunks laid out in ORDER)
        tok_off = [0] * NCHUNK
        for oi, c in enumerate(ORDER):
            tok_off[c] = tok0 + oi * H

        def proj(oi, c):
            """u_c = tokT_c @ w columns, accumulated on TensorE."""
            ps = psp.tile([P, nw_u], F32, name=f"u_ps_{oi}")
            for j in range(NBLK):
                if general:
                    rhs = blob_sb[:, w0 + 2 * j:w0 + 2 * j + 2]
                elif c < N_SRC_CHUNKS:
                    rhs = blob_sb[:, w0 + 2 * j:w0 + 2 * j + 1]
                else:
                    rhs = blob_sb[:, w0 + 2 * j + 1:w0 + 2 * j + 2]
                t0 = tok_off[c] + j * P
                nc.tensor.matmul(ps[:], blob_sb[:, t0:t0 + P], rhs,
                                 start=(j == 0), stop=(j == NBLK - 1))
            nc.vector.tensor_copy(u_sb[:, oi * nw_u:(oi + 1) * nw_u], ps[:])

        mm_i = 0

        def pool(oi, c):
            """Accumulate this chunk's pooling + outer-sum into psum_out."""
            nonlocal mm_i
            n_mm = 2 * NCHUNK if general else NCHUNK
            sides = ((0, True), (1, False)) if general else (
                ((0, True),) if c < N_SRC_CHUNKS else ((0, False),))
            for ui, is_src in sides:
                ucol = u_sb[:, oi * nw_u + ui:oi * nw_u + ui + 1]
                first = mm_i == 0
                last = mm_i == n_mm - 1
                if general:
                    a = atw[oi][:, 0:S] if is_src else atw[oi][:, S:S + T]
                else:
                    a = atw[oi][:]
                if is_src:
                    nc.tensor.matmul(psum_out[:], a, ucol.broadcast_to([P, T]),
                                     start=first, stop=last)
                else:
                    nc.tensor.matmul(psum_out[:], ucol.broadcast_to([P, S]), a,
                                     start=first, stop=last)
                mm_i += 1

        # emission order == per-engine execution order: projections lead
        # their pooling matmuls so the DVE u-copies hide under TensorE work
        proj(0, ORDER[0])
        proj(1, ORDER[1])
        pool(0, ORDER[0])
        pool(1, ORDER[1])
        proj(2, ORDER[2])
        proj(3, ORDER[3])
        pool(2, ORDER[2])
        pool(3, ORDER[3])

        # one bias-add copy PSUM->SBUF, then both ring engines issue half
        # the store each in parallel (both wait on the same DVE op)
        out_sb = sbp.tile([S, T], F32)
        hs = S // 2
        nc.vector.tensor_scalar_add(out_sb[:], psum_out[:], meta_f[0:S, 0:1])
        nc.sync.dma_start(out=out_d[0:hs, :], in_=out_sb[0:hs, :])
        nc.scalar.dma_start(out=out_d[hs:S, :], in_=out_sb[hs:S, :])


@functools.lru_cache(maxsize=4)
def _build(S, T, block_ok):
    nc = bacc.Bacc("TRN2", debug=False, num_devices=N_CORES)
    with TileContext(nc) as tc:
        _emit_body(nc, tc, S, T, P if block_ok else S + T)
    nc.compile()
    return nc


# ---------------------------------------------------------------------------
# Host wrapper
# ---------------------------------------------------------------------------

def _prep(inputs):
    tok_h = np.ascontiguousarray(np.asarray(inputs["tok_h"], dtype=np.float32))
    mask = np.asarray(inputs["attention_mask"])
    swid = np.asarray(inputs["source_word_ids"])
    twid = np.asarray(inputs["target_word_ids"])
    W = np.asarray(inputs["W"], dtype=np.float32)
    b = np.asarray(inputs["b"], dtype=np.float32)
    S = int(np.asarray(inputs["S"]))
    T = int(np.asarray(inputs["T"]))

    Bv, Lv, Hv = tok_h.shape
    assert (Bv, Lv, Hv) == (B, L, H), f"unexpected tok_h shape {tok_h.shape}"
    assert swid.shape == (B, L_SRC) and twid.shape == (B, L_TGT)
    assert S <= P and T <= P

    NW = S + T
    combined = np.concatenate([swid, twid], axis=1).astype(np.int64)
    seg, valid = _segments(combined, mask, NW)
    wgt = _seg_weights(seg, valid, NW)

    src_tok_seg = seg[:, :L_SRC][valid[:, :L_SRC]]
    tgt_tok_seg = seg[:, L_SRC:][valid[:, L_SRC:]]
    block_ok = bool(
        (src_tok_seg < S).all()
        and (tgt_tok_seg >= S).all() and (tgt_tok_seg < NW).all()
    )
    aw = P if block_ok else NW
    w0, iota0, meta0, tok0, ncol = _cols(aw)

    # per-token pooling column index (+weight); invalid tokens never match
    use_col = seg.astype(np.int64)
    invalid = ~(valid & (seg < NW))
    if block_ok:
        use_col = use_col - np.where(use_col >= S, S, 0)
    use_col = np.where(invalid, -1, use_col)

    # tokT[b, p, oi*768 + j*128 + t] = tok_h[b, ORDER[oi]*128 + t, j*128 + p]
    tok6 = tok_h.reshape(B, NCHUNK, P, NBLK, P)        # [b, c, t, j, p]
    tokT = tok6[:, ORDER].transpose(0, 4, 1, 3, 2).reshape(B, P, NCHUNK * H)

    blob = np.zeros((B, P, ncol), dtype=np.float32)
    blob[:, :, tok0:ncol] = tokT                       # chunks in ORDER
    blob[:, :, w0:w0 + NW_META:2] = W[:H, 0].reshape(NBLK, P).T[None]
    blob[:, :, w0 + 1:w0 + NW_META:2] = W[H:2 * H, 0].reshape(NBLK, P).T[None]
    blob[:, :, iota0:iota0 + aw] = np.arange(aw, dtype=np.float32)[None, None]
    blob[:, :, meta0] = b.reshape(-1)[0]
    for oi, c in enumerate(ORDER):
        blob[:, :, meta0 + 1 + oi] = use_col[:, c * P:(c + 1) * P]
        blob[:, :, meta0 + 5 + oi] = wgt[:, c * P:(c + 1) * P]
    blob = np.ascontiguousarray(blob.astype(ml_dtypes.bfloat16))

    in_maps = [{"blob": blob[i % B]} for i in range(N_CORES)]
    return S, T, block_ok, in_maps


def kernel(**inputs):
    S, T, block_ok, in_maps = _prep(inputs)
    nc = _build(S, T, block_ok)
    res = run_bass_kernel_spmd(nc, in_maps, core_ids=list(range(N_CORES)))
    return np.stack([res.results[i]["out"] for i in range(B)], axis=0)
